# revision 4
# baseline (speedup 1.0000x reference)
"""Trainium2 Bass kernel for the NeuroPlasticityMechanism problem.

Row-sharded tensor-parallel across 8 NeuronCores (no communication):
each core gets a contiguous row-block of every large tensor plus the full
(small) V factors.  Per element the kernel computes

    t' = 0.9*t + 0.1*g
    m' = 0.95*m + 0.05*t'
    mod = (0.1*reward)*m' + 0.1*(U@V)        (matrix params)
    mod = (0.1*reward)*m'                    (bias)

streaming [128 x 2048] fp32 tiles: 2 ScalarE scale ops + 3 VectorE fused
scalar_tensor_tensor ops per tile, U@V on the PE from host-pretransposed
U^T slices, everything overlapped with HWDGE DMA.  The kernel is
HBM-bandwidth-bound (~109 MB of DMA traffic per core).
"""
import sys
import types
from contextlib import ExitStack

import numpy as np

D, F, R = 2048, 8192, 64
NCORES = 8
RA = D // NCORES      # 256 attn rows per core
R1 = F // NCORES      # 1024 ff1 rows per core
R2 = D // NCORES      # 256 ff2 rows per core
RB = F // NCORES      # 1024 bias elems per core
TILE_N = 2048         # free-dim stream tile
MM_N = 512            # fp32 matmul moving-operand max / one PSUM bank


def _install_tile_drain_patch():
    """This walrus build caps sync-wait commands on TPB_CTRL (Drain)
    instructions below what Tile's tail drain emits.  Re-emit the waits as
    individual wait_ge instructions on the sync engine, then a bare drain."""
    from concourse import tile
    from concourse.vector_clock import ScopedClock

    if getattr(tile.TileContext, "_drain_patch_installed", False):
        return

    def _drain_and_barrier(self, tick_clock, wait_clock):
        nc = self.nc
        probe = nc.sync.nop(nofuse=True)
        wait_clock.add_sem_waits(
            probe.ins, ScopedClock({None: tick_clock.global_clock})
        )
        si = probe.ins.sync_info
        waits = list(si.on_wait or []) if si else []
        if si:
            si.on_wait = []
        handles = {h.name: h for h in self.sems.allocated().values()}
        for w in waits:
            assert w.wait_mode == "sem-ge-imm", w
            nc.sync.wait_ge(handles[w.ant_name], w.wait_value)
        nc.sync.drain()
        nc.all_engine_barrier()
        popped = nc._tile_sem_poison_stack.pop()
        assert popped is self._sem_poison
        nc.clear_and_free_semaphores(list(self.sems.allocated().values()))
        nc.all_engine_barrier()

    tile.TileContext._drain_and_barrier = _drain_and_barrier
    tile.TileContext._drain_patch_installed = True


def _split_multi_waits(nc):
    """This stack's walrus accepts only one sync-wait command per
    instruction, but Tile's scheduler attaches several.  Hoist all but the
    last wait of each instruction onto dedicated EventSemaphore wait
    instructions inserted just before it on the same engine queue."""
    from concourse import mybir

    n = 0
    for f in nc.m.functions:
        for bb in f.blocks:
            out = []
            changed = False
            for inst in bb.instructions:
                si = inst.sync_info
                if si is not None and si.on_wait and len(si.on_wait) > 1:
                    waits = list(si.on_wait)
                    for w in waits[:-1]:
                        n += 1
                        out.append(mybir.InstEventSemaphore(
                            name=f"I-waitsplit-{n}", ins=[], outs=[],
                            engine=inst.engine,
                            sync_info=mybir.SyncInfo(on_wait=[w], on_update=[]),
                        ))
                    si.on_wait = [waits[-1]]
                    changed = True
                out.append(inst)
            if changed:
                bb.instructions = out


_BUILT = {}


def _build_program():
    if "nc" in _BUILT:
        return _BUILT["nc"]

    _install_tile_drain_patch()
    import concourse.bass as bass
    import concourse.tile as tile
    from concourse import mybir

    f32 = mybir.dt.float32
    MULT = mybir.AluOpType.mult
    ADD = mybir.AluOpType.add

    nc = bass.Bass()

    def inp(name, shape):
        return nc.declare_dram_parameter(name, list(shape), f32, isOutput=False)

    def outp(name, shape):
        return nc.declare_dram_parameter(name, list(shape), f32, isOutput=True)

    ga, ta, ma = (inp(n, (RA, D)) for n in ("ga", "ta", "ma"))
    uta = inp("uta", (R, RA))          # U_attn shard, pre-transposed
    va = inp("va", (R, D))
    g1, t1, m1 = (inp(n, (R1, D)) for n in ("g1", "t1", "m1"))
    ut1 = inp("ut1", (R, R1))
    v1 = inp("v1", (R, D))
    g2, t2, m2 = (inp(n, (R2, F)) for n in ("g2", "t2", "m2"))
    ut2 = inp("ut2", (R, R2))
    v2 = inp("v2", (R, F))
    gb, tb, mb = (inp(n, (8, 128)) for n in ("gb", "tb", "mb"))
    rw = inp("rw", (128, 1))

    ota_t, ota_m, ota_d = (outp(n, (RA, D)) for n in ("ota_t", "ota_m", "ota_d"))
    o1_t, o1_m, o1_d = (outp(n, (R1, D)) for n in ("o1_t", "o1_m", "o1_d"))
    o2_t, o2_m, o2_d = (outp(n, (R2, F)) for n in ("o2_t", "o2_m", "o2_d"))
    ob_t, ob_m, ob_d = (outp(n, (8, 128)) for n in ("ob_t", "ob_m", "ob_d"))

    with tile.TileContext(nc) as tc, ExitStack() as ctx:
        const = ctx.enter_context(tc.tile_pool(name="const", bufs=1))

        s_t = const.tile([128, 1], f32)
        nc.sync.dma_start(out=s_t[:], in_=rw[:])
        nc.vector.tensor_scalar_mul(s_t[:], s_t[:], 0.1)   # s = 0.1 * reward

        va_t = const.tile([R, D], f32)
        nc.sync.dma_start(out=va_t[:], in_=va[:])
        nc.gpsimd.tensor_scalar_mul(va_t[:], va_t[:], 0.1)  # 0.1 * V folded in
        v1_t = const.tile([R, D], f32)
        nc.sync.dma_start(out=v1_t[:], in_=v1[:])
        nc.gpsimd.tensor_scalar_mul(v1_t[:], v1_t[:], 0.1)
        v2_t = const.tile([R, F], f32)
        nc.sync.dma_start(out=v2_t[:], in_=v2[:])
        nc.gpsimd.tensor_scalar_mul(v2_t[:], v2_t[:], 0.1)

        uta_t = const.tile([R, RA], f32)
        nc.sync.dma_start(out=uta_t[:], in_=uta[:])
        ut1_t = const.tile([R, R1], f32)
        nc.sync.dma_start(out=ut1_t[:], in_=ut1[:])
        ut2_t = const.tile([R, R2], f32)
        nc.sync.dma_start(out=ut2_t[:], in_=ut2[:])

        gp = ctx.enter_context(tc.tile_pool(name="gp", bufs=4))
        tp = ctx.enter_context(tc.tile_pool(name="tp", bufs=4))
        mp = ctx.enter_context(tc.tile_pool(name="mp", bufs=4))
        dp = ctx.enter_context(tc.tile_pool(name="dp", bufs=4))
        pp = ctx.enter_context(tc.tile_pool(name="pp", bufs=2, space="PSUM"))

        def position(gd, td, md, r0, c0, ut_t, v_t, od_t, od_m, od_d):
            rs, cs = slice(r0, r0 + 128), slice(c0, c0 + TILE_N)
            gt = gp.tile([128, TILE_N], f32)
            nc.sync.dma_start(out=gt[:], in_=gd[rs, cs])
            tt = tp.tile([128, TILE_N], f32)
            nc.sync.dma_start(out=tt[:], in_=td[rs, cs])
            mt = mp.tile([128, TILE_N], f32)
            nc.sync.dma_start(out=mt[:], in_=md[rs, cs])

            nc.scalar.mul(gt[:], gt[:], 0.1)                       # 0.1*g
            nc.vector.scalar_tensor_tensor(                        # t' = 0.9t + 0.1g
                tt[:], tt[:], 0.9, gt[:], op0=MULT, op1=ADD)
            nc.scalar.dma_start(out=od_t[rs, cs], in_=tt[:])
            nc.scalar.mul(gt[:], tt[:], 0.05)                      # 0.05*t'
            nc.vector.scalar_tensor_tensor(                        # m' = 0.95m + 0.05t'
                mt[:], mt[:], 0.95, gt[:], op0=MULT, op1=ADD)
            nc.scalar.dma_start(out=od_m[rs, cs], in_=mt[:])

            ps = pp.tile([128, TILE_N], f32)
            for j in range(TILE_N // MM_N):
                nc.tensor.matmul(                                  # 0.1*(U@V) chunk
                    ps[:, j * MM_N:(j + 1) * MM_N],
                    lhsT=ut_t[:, r0:r0 + 128],
                    rhs=v_t[:, c0 + j * MM_N:c0 + (j + 1) * MM_N],
                    start=True, stop=True)
            dt = dp.tile([128, TILE_N], f32)
            nc.vector.scalar_tensor_tensor(                        # mod = s*m' + UV
                dt[:], mt[:], s_t[:, 0:1], ps[:], op0=MULT, op1=ADD)
            nc.scalar.dma_start(out=od_d[rs, cs], in_=dt[:])

        for r0 in range(0, RA, 128):
            position(ga, ta, ma, r0, 0, uta_t, va_t, ota_t, ota_m, ota_d)
        for r0 in range(0, R1, 128):
            position(g1, t1, m1, r0, 0, ut1_t, v1_t, o1_t, o1_m, o1_d)
        for r0 in range(0, R2, 128):
            for c0 in range(0, F, TILE_N):
                position(g2, t2, m2, r0, c0, ut2_t, v2_t, o2_t, o2_m, o2_d)

        # bias shard [8, 128] -- no low-rank term
        gt = gp.tile([8, 128], f32, tag="gt")
        nc.sync.dma_start(out=gt[:], in_=gb[:])
        tt = tp.tile([8, 128], f32, tag="tt")
        nc.sync.dma_start(out=tt[:], in_=tb[:])
        mt = mp.tile([8, 128], f32, tag="mt")
        nc.sync.dma_start(out=mt[:], in_=mb[:])
        nc.scalar.mul(gt[:], gt[:], 0.1)
        nc.vector.scalar_tensor_tensor(tt[:], tt[:], 0.9, gt[:], op0=MULT, op1=ADD)
        nc.scalar.dma_start(out=ob_t[:], in_=tt[:])
        nc.scalar.mul(gt[:], tt[:], 0.05)
        nc.vector.scalar_tensor_tensor(mt[:], mt[:], 0.95, gt[:], op0=MULT, op1=ADD)
        nc.scalar.dma_start(out=ob_m[:], in_=mt[:])
        dt = dp.tile([8, 128], f32, tag="dt")
        nc.vector.tensor_scalar_mul(dt[:], mt[:], s_t[0:8, 0:1])
        nc.scalar.dma_start(out=ob_d[:], in_=dt[:])

    _split_multi_waits(nc)
    _BUILT["nc"] = nc
    return nc


def _shard_inputs(inputs):
    """Full inputs -> one in_map per core (row-sharded, views where possible)."""
    c32 = lambda a: np.ascontiguousarray(a, dtype=np.float32)
    rw = c32(np.broadcast_to(inputs["reward_signal"].reshape(1, 1), (128, 1)))
    in_maps = []
    for c in range(NCORES):
        sa = slice(c * RA, (c + 1) * RA)
        s1 = slice(c * R1, (c + 1) * R1)
        s2 = slice(c * R2, (c + 1) * R2)
        sb = slice(c * RB, (c + 1) * RB)
        in_maps.append({
            "ga": inputs["grad_attn"][sa], "ta": inputs["trace_attn"][sa],
            "ma": inputs["mom_attn"][sa],
            "uta": c32(inputs["U_attn"][sa].T), "va": inputs["V_attn"],
            "g1": inputs["grad_ff1"][s1], "t1": inputs["trace_ff1"][s1],
            "m1": inputs["mom_ff1"][s1],
            "ut1": c32(inputs["U_ff1"][s1].T), "v1": inputs["V_ff1"],
            "g2": inputs["grad_ff2"][s2], "t2": inputs["trace_ff2"][s2],
            "m2": inputs["mom_ff2"][s2],
            "ut2": c32(inputs["U_ff2"][s2].T), "v2": inputs["V_ff2"],
            "gb": inputs["grad_b"][sb].reshape(8, 128),
            "tb": inputs["trace_b"][sb].reshape(8, 128),
            "mb": inputs["mom_b"][sb].reshape(8, 128),
            "rw": rw,
        })
    return in_maps


def _gather(results):
    cat = lambda key: np.concatenate([results[c][key] for c in range(NCORES)], axis=0)
    catb = lambda key: np.concatenate(
        [results[c][key].reshape(RB) for c in range(NCORES)], axis=0)
    return (
        cat("ota_t"), cat("ota_m"), cat("ota_d"),
        cat("o1_t"), cat("o1_m"), cat("o1_d"),
        cat("o2_t"), cat("o2_m"), cat("o2_d"),
        catb("ob_t"), catb("ob_m"), catb("ob_d"),
    )


def run_sharded(inputs, trace=False, tmpdir=None):
    """Build + run; returns (outputs_tuple, BassKernelResults)."""
    nc = _build_program()
    import concourse.bass_utils as bass_utils
    in_maps = _shard_inputs(inputs)
    res = bass_utils.run_bass_kernel_spmd(
        nc, in_maps, list(range(NCORES)), trace=trace, tmpdir=tmpdir)
    return _gather(res.results), res


def kernel(**inputs):
    outputs, _ = run_sharded(inputs, trace=False)
    return outputs


# revision 5
# speedup vs baseline: 1.2513x; 1.2513x over previous
"""Trainium2 Bass kernel for the NeuroPlasticityMechanism problem.

Row-sharded tensor-parallel across 8 NeuronCores (no communication):
each core gets a contiguous row-block of every large tensor plus the full
(small) V factors.  Per element the kernel computes

    t' = 0.9*t + 0.1*g
    m' = 0.95*m + 0.05*t'
    mod = (0.1*reward)*m' + 0.1*(U@V)        (matrix params)
    mod = (0.1*reward)*m'                    (bias)

streaming [128 x 2048] fp32 tiles: 2 ScalarE scale ops + 3 VectorE fused
scalar_tensor_tensor ops per tile, U@V on the PE from host-pretransposed
U^T slices, everything overlapped with HWDGE DMA.  The kernel is
HBM-bandwidth-bound (~109 MB of DMA traffic per core).
"""
import sys
import types
from contextlib import ExitStack

import numpy as np

D, F, R = 2048, 8192, 64
NCORES = 8
RA = D // NCORES      # 256 attn rows per core
R1 = F // NCORES      # 1024 ff1 rows per core
R2 = D // NCORES      # 256 ff2 rows per core
RB = F // NCORES      # 1024 bias elems per core
TILE_N = 2048         # free-dim stream tile
MM_N = 512            # fp32 matmul moving-operand max / one PSUM bank


def _install_tile_drain_patch():
    """This walrus build caps sync-wait commands on TPB_CTRL (Drain)
    instructions below what Tile's tail drain emits.  Re-emit the waits as
    individual wait_ge instructions on the sync engine, then a bare drain."""
    from concourse import tile
    from concourse.vector_clock import ScopedClock

    if getattr(tile.TileContext, "_drain_patch_installed", False):
        return

    def _drain_and_barrier(self, tick_clock, wait_clock):
        nc = self.nc
        probe = nc.sync.nop(nofuse=True)
        wait_clock.add_sem_waits(
            probe.ins, ScopedClock({None: tick_clock.global_clock})
        )
        si = probe.ins.sync_info
        waits = list(si.on_wait or []) if si else []
        if si:
            si.on_wait = []
        handles = {h.name: h for h in self.sems.allocated().values()}
        for w in waits:
            assert w.wait_mode == "sem-ge-imm", w
            nc.sync.wait_ge(handles[w.ant_name], w.wait_value)
        nc.sync.drain()
        nc.all_engine_barrier()
        popped = nc._tile_sem_poison_stack.pop()
        assert popped is self._sem_poison
        nc.clear_and_free_semaphores(list(self.sems.allocated().values()))
        nc.all_engine_barrier()

    tile.TileContext._drain_and_barrier = _drain_and_barrier
    tile.TileContext._drain_patch_installed = True


def _split_multi_waits(nc):
    """This stack's walrus accepts only one sync-wait command per
    instruction, but Tile's scheduler attaches several.  Hoist all but the
    last wait of each instruction onto dedicated EventSemaphore wait
    instructions inserted just before it on the same engine queue."""
    from concourse import mybir

    n = 0
    for f in nc.m.functions:
        for bb in f.blocks:
            out = []
            changed = False
            for inst in bb.instructions:
                si = inst.sync_info
                if si is not None and si.on_wait and len(si.on_wait) > 1:
                    waits = list(si.on_wait)
                    for w in waits[:-1]:
                        n += 1
                        out.append(mybir.InstEventSemaphore(
                            name=f"I-waitsplit-{n}", ins=[], outs=[],
                            engine=inst.engine,
                            sync_info=mybir.SyncInfo(on_wait=[w], on_update=[]),
                        ))
                    si.on_wait = [waits[-1]]
                    changed = True
                out.append(inst)
            if changed:
                bb.instructions = out


_BUILT = {}


def _build_program():
    if "nc" in _BUILT:
        return _BUILT["nc"]

    _install_tile_drain_patch()
    import concourse.bass as bass
    import concourse.tile as tile
    from concourse import mybir

    f32 = mybir.dt.float32
    MULT = mybir.AluOpType.mult
    ADD = mybir.AluOpType.add

    nc = bass.Bass()

    def inp(name, shape):
        return nc.declare_dram_parameter(name, list(shape), f32, isOutput=False)

    def outp(name, shape):
        return nc.declare_dram_parameter(name, list(shape), f32, isOutput=True)

    ga, ta, ma = (inp(n, (RA, D)) for n in ("ga", "ta", "ma"))
    uta = inp("uta", (R, RA))          # U_attn shard, pre-transposed
    va = inp("va", (R, D))
    g1, t1, m1 = (inp(n, (R1, D)) for n in ("g1", "t1", "m1"))
    ut1 = inp("ut1", (R, R1))
    v1 = inp("v1", (R, D))
    g2, t2, m2 = (inp(n, (R2, F)) for n in ("g2", "t2", "m2"))
    ut2 = inp("ut2", (R, R2))
    v2 = inp("v2", (R, F))
    gb, tb, mb = (inp(n, (8, 128)) for n in ("gb", "tb", "mb"))
    rw = inp("rw", (128, 1))

    ota_t, ota_m, ota_d = (outp(n, (RA, D)) for n in ("ota_t", "ota_m", "ota_d"))
    o1_t, o1_m, o1_d = (outp(n, (R1, D)) for n in ("o1_t", "o1_m", "o1_d"))
    o2_t, o2_m, o2_d = (outp(n, (R2, F)) for n in ("o2_t", "o2_m", "o2_d"))
    ob_t, ob_m, ob_d = (outp(n, (8, 128)) for n in ("ob_t", "ob_m", "ob_d"))

    with tile.TileContext(nc) as tc, ExitStack() as ctx:
        const = ctx.enter_context(tc.tile_pool(name="const", bufs=1))

        s_t = const.tile([128, 1], f32)
        nc.sync.dma_start(out=s_t[:], in_=rw[:])
        nc.vector.tensor_scalar_mul(s_t[:], s_t[:], 0.1)   # s = 0.1 * reward

        va_t = const.tile([R, D], f32)
        nc.sync.dma_start(out=va_t[:], in_=va[:])
        nc.scalar.mul(va_t[:], va_t[:], 0.1)               # 0.1 * V folded in
        v1_t = const.tile([R, D], f32)
        nc.sync.dma_start(out=v1_t[:], in_=v1[:])
        nc.scalar.mul(v1_t[:], v1_t[:], 0.1)
        v2_t = const.tile([R, F], f32)
        nc.sync.dma_start(out=v2_t[:], in_=v2[:])
        nc.scalar.mul(v2_t[:], v2_t[:], 0.1)

        uta_t = const.tile([R, RA], f32)
        nc.sync.dma_start(out=uta_t[:], in_=uta[:])
        ut1_t = const.tile([R, R1], f32)
        nc.sync.dma_start(out=ut1_t[:], in_=ut1[:])
        ut2_t = const.tile([R, R2], f32)
        nc.sync.dma_start(out=ut2_t[:], in_=ut2[:])

        gp = ctx.enter_context(tc.tile_pool(name="gp", bufs=4))
        tp = ctx.enter_context(tc.tile_pool(name="tp", bufs=4))
        mp = ctx.enter_context(tc.tile_pool(name="mp", bufs=4))
        dp = ctx.enter_context(tc.tile_pool(name="dp", bufs=4))
        pp = ctx.enter_context(tc.tile_pool(name="pp", bufs=2, space="PSUM"))

        def position(gd, td, md, r0, c0, ut_t, v_t, od_t, od_m, od_d):
            rs, cs = slice(r0, r0 + 128), slice(c0, c0 + TILE_N)
            gt = gp.tile([128, TILE_N], f32)
            nc.sync.dma_start(out=gt[:], in_=gd[rs, cs])
            tt = tp.tile([128, TILE_N], f32)
            nc.sync.dma_start(out=tt[:], in_=td[rs, cs])
            mt = mp.tile([128, TILE_N], f32)
            nc.sync.dma_start(out=mt[:], in_=md[rs, cs])

            nc.scalar.mul(gt[:], gt[:], 0.1)                       # 0.1*g
            nc.vector.scalar_tensor_tensor(                        # t' = 0.9t + 0.1g
                tt[:], tt[:], 0.9, gt[:], op0=MULT, op1=ADD)
            nc.scalar.dma_start(out=od_t[rs, cs], in_=tt[:])
            nc.scalar.mul(gt[:], tt[:], 0.05)                      # 0.05*t'
            nc.vector.scalar_tensor_tensor(                        # m' = 0.95m + 0.05t'
                mt[:], mt[:], 0.95, gt[:], op0=MULT, op1=ADD)
            nc.scalar.dma_start(out=od_m[rs, cs], in_=mt[:])

            ps = pp.tile([128, TILE_N], f32)
            for j in range(TILE_N // MM_N):
                nc.tensor.matmul(                                  # 0.1*(U@V) chunk
                    ps[:, j * MM_N:(j + 1) * MM_N],
                    lhsT=ut_t[:, r0:r0 + 128],
                    rhs=v_t[:, c0 + j * MM_N:c0 + (j + 1) * MM_N],
                    start=True, stop=True)
            dt = dp.tile([128, TILE_N], f32)
            nc.vector.scalar_tensor_tensor(                        # mod = s*m' + UV
                dt[:], mt[:], s_t[:, 0:1], ps[:], op0=MULT, op1=ADD)
            nc.scalar.dma_start(out=od_d[rs, cs], in_=dt[:])

        for r0 in range(0, RA, 128):
            position(ga, ta, ma, r0, 0, uta_t, va_t, ota_t, ota_m, ota_d)
        for r0 in range(0, R1, 128):
            position(g1, t1, m1, r0, 0, ut1_t, v1_t, o1_t, o1_m, o1_d)
        for r0 in range(0, R2, 128):
            for c0 in range(0, F, TILE_N):
                position(g2, t2, m2, r0, c0, ut2_t, v2_t, o2_t, o2_m, o2_d)

        # bias shard [8, 128] -- no low-rank term
        gt = gp.tile([8, 128], f32, tag="gt")
        nc.sync.dma_start(out=gt[:], in_=gb[:])
        tt = tp.tile([8, 128], f32, tag="tt")
        nc.sync.dma_start(out=tt[:], in_=tb[:])
        mt = mp.tile([8, 128], f32, tag="mt")
        nc.sync.dma_start(out=mt[:], in_=mb[:])
        nc.scalar.mul(gt[:], gt[:], 0.1)
        nc.vector.scalar_tensor_tensor(tt[:], tt[:], 0.9, gt[:], op0=MULT, op1=ADD)
        nc.scalar.dma_start(out=ob_t[:], in_=tt[:])
        nc.scalar.mul(gt[:], tt[:], 0.05)
        nc.vector.scalar_tensor_tensor(mt[:], mt[:], 0.95, gt[:], op0=MULT, op1=ADD)
        nc.scalar.dma_start(out=ob_m[:], in_=mt[:])
        dt = dp.tile([8, 128], f32, tag="dt")
        nc.vector.tensor_scalar_mul(dt[:], mt[:], s_t[0:8, 0:1])
        nc.scalar.dma_start(out=ob_d[:], in_=dt[:])

    _split_multi_waits(nc)
    _BUILT["nc"] = nc
    return nc


def _shard_inputs(inputs):
    """Full inputs -> one in_map per core (row-sharded, views where possible)."""
    c32 = lambda a: np.ascontiguousarray(a, dtype=np.float32)
    rw = c32(np.broadcast_to(inputs["reward_signal"].reshape(1, 1), (128, 1)))
    in_maps = []
    for c in range(NCORES):
        sa = slice(c * RA, (c + 1) * RA)
        s1 = slice(c * R1, (c + 1) * R1)
        s2 = slice(c * R2, (c + 1) * R2)
        sb = slice(c * RB, (c + 1) * RB)
        in_maps.append({
            "ga": inputs["grad_attn"][sa], "ta": inputs["trace_attn"][sa],
            "ma": inputs["mom_attn"][sa],
            "uta": c32(inputs["U_attn"][sa].T), "va": inputs["V_attn"],
            "g1": inputs["grad_ff1"][s1], "t1": inputs["trace_ff1"][s1],
            "m1": inputs["mom_ff1"][s1],
            "ut1": c32(inputs["U_ff1"][s1].T), "v1": inputs["V_ff1"],
            "g2": inputs["grad_ff2"][s2], "t2": inputs["trace_ff2"][s2],
            "m2": inputs["mom_ff2"][s2],
            "ut2": c32(inputs["U_ff2"][s2].T), "v2": inputs["V_ff2"],
            "gb": inputs["grad_b"][sb].reshape(8, 128),
            "tb": inputs["trace_b"][sb].reshape(8, 128),
            "mb": inputs["mom_b"][sb].reshape(8, 128),
            "rw": rw,
        })
    return in_maps


def _gather(results):
    cat = lambda key: np.concatenate([results[c][key] for c in range(NCORES)], axis=0)
    catb = lambda key: np.concatenate(
        [results[c][key].reshape(RB) for c in range(NCORES)], axis=0)
    return (
        cat("ota_t"), cat("ota_m"), cat("ota_d"),
        cat("o1_t"), cat("o1_m"), cat("o1_d"),
        cat("o2_t"), cat("o2_m"), cat("o2_d"),
        catb("ob_t"), catb("ob_m"), catb("ob_d"),
    )


def run_sharded(inputs, trace=False, tmpdir=None):
    """Build + run; returns (outputs_tuple, BassKernelResults)."""
    nc = _build_program()
    import concourse.bass_utils as bass_utils
    in_maps = _shard_inputs(inputs)
    res = bass_utils.run_bass_kernel_spmd(
        nc, in_maps, list(range(NCORES)), trace=trace, tmpdir=tmpdir)
    return _gather(res.results), res


def kernel(**inputs):
    outputs, _ = run_sharded(inputs, trace=False)
    return outputs


# revision 6
# speedup vs baseline: 1.4211x; 1.1357x over previous
"""Trainium2 Bass kernel for the NeuroPlasticityMechanism problem.

Row-sharded tensor-parallel across 8 NeuronCores (no communication):
each core gets a contiguous row-block of every large tensor plus the full
(small) V factors.  Per element the kernel computes

    t' = 0.9*t + 0.1*g
    m' = 0.95*m + 0.05*t'
    mod = (0.1*reward)*m' + 0.1*(U@V)        (matrix params)
    mod = (0.1*reward)*m'                    (bias)

streaming [128 x 2048] fp32 tiles: 2 ScalarE scale ops + 3 VectorE fused
scalar_tensor_tensor ops per tile, U@V on the PE from host-pretransposed
U^T slices, everything overlapped with HWDGE DMA.  The kernel is
HBM-bandwidth-bound (~109 MB of DMA traffic per core).
"""
import sys
import types
from contextlib import ExitStack

import numpy as np

D, F, R = 2048, 8192, 64
NCORES = 8
RA = D // NCORES      # 256 attn rows per core
R1 = F // NCORES      # 1024 ff1 rows per core
R2 = D // NCORES      # 256 ff2 rows per core
RB = F // NCORES      # 1024 bias elems per core
TILE_N = 2048         # free-dim stream tile
MM_N = 512            # fp32 matmul moving-operand max / one PSUM bank


def _install_tile_drain_patch():
    """This walrus build caps sync-wait commands on TPB_CTRL (Drain)
    instructions below what Tile's tail drain emits.  Re-emit the waits as
    individual wait_ge instructions on the sync engine, then a bare drain."""
    from concourse import tile
    from concourse.vector_clock import ScopedClock

    if getattr(tile.TileContext, "_drain_patch_installed", False):
        return

    def _drain_and_barrier(self, tick_clock, wait_clock):
        nc = self.nc
        probe = nc.sync.nop(nofuse=True)
        wait_clock.add_sem_waits(
            probe.ins, ScopedClock({None: tick_clock.global_clock})
        )
        si = probe.ins.sync_info
        waits = list(si.on_wait or []) if si else []
        if si:
            si.on_wait = []
        handles = {h.name: h for h in self.sems.allocated().values()}
        for w in waits:
            assert w.wait_mode == "sem-ge-imm", w
            nc.sync.wait_ge(handles[w.ant_name], w.wait_value)
        nc.sync.drain()
        nc.all_engine_barrier()
        popped = nc._tile_sem_poison_stack.pop()
        assert popped is self._sem_poison
        nc.clear_and_free_semaphores(list(self.sems.allocated().values()))
        nc.all_engine_barrier()

    tile.TileContext._drain_and_barrier = _drain_and_barrier
    tile.TileContext._drain_patch_installed = True


def _split_multi_waits(nc):
    """This stack's walrus accepts only one sync-wait command per
    instruction, but Tile's scheduler attaches several.  Hoist all but the
    last wait of each instruction onto dedicated EventSemaphore wait
    instructions inserted just before it on the same engine queue."""
    from concourse import mybir

    n = 0
    for f in nc.m.functions:
        for bb in f.blocks:
            out = []
            changed = False
            for inst in bb.instructions:
                si = inst.sync_info
                if si is not None and si.on_wait and len(si.on_wait) > 1:
                    waits = list(si.on_wait)
                    for w in waits[:-1]:
                        n += 1
                        out.append(mybir.InstEventSemaphore(
                            name=f"I-waitsplit-{n}", ins=[], outs=[],
                            engine=inst.engine,
                            sync_info=mybir.SyncInfo(on_wait=[w], on_update=[]),
                        ))
                    si.on_wait = [waits[-1]]
                    changed = True
                out.append(inst)
            if changed:
                bb.instructions = out


_BUILT = {}


def _build_program():
    if "nc" in _BUILT:
        return _BUILT["nc"]

    _install_tile_drain_patch()
    import concourse.bass as bass
    import concourse.tile as tile
    from concourse import mybir

    f32 = mybir.dt.float32
    MULT = mybir.AluOpType.mult
    ADD = mybir.AluOpType.add

    nc = bass.Bass()

    def inp(name, shape):
        return nc.declare_dram_parameter(name, list(shape), f32, isOutput=False)

    def outp(name, shape):
        return nc.declare_dram_parameter(name, list(shape), f32, isOutput=True)

    ga, ta, ma = (inp(n, (RA, D)) for n in ("ga", "ta", "ma"))
    uta = inp("uta", (R, RA))          # U_attn shard, pre-transposed
    va = inp("va", (R, D))
    g1, t1, m1 = (inp(n, (R1, D)) for n in ("g1", "t1", "m1"))
    ut1 = inp("ut1", (R, R1))
    v1 = inp("v1", (R, D))
    g2, t2, m2 = (inp(n, (R2, F)) for n in ("g2", "t2", "m2"))
    ut2 = inp("ut2", (R, R2))
    v2 = inp("v2", (R, F))
    gb, tb, mb = (inp(n, (8, 128)) for n in ("gb", "tb", "mb"))
    rw = inp("rw", (128, 1))

    ota_t, ota_m, ota_d = (outp(n, (RA, D)) for n in ("ota_t", "ota_m", "ota_d"))
    o1_t, o1_m, o1_d = (outp(n, (R1, D)) for n in ("o1_t", "o1_m", "o1_d"))
    o2_t, o2_m, o2_d = (outp(n, (R2, F)) for n in ("o2_t", "o2_m", "o2_d"))
    ob_t, ob_m, ob_d = (outp(n, (8, 128)) for n in ("ob_t", "ob_m", "ob_d"))

    with tile.TileContext(nc) as tc, ExitStack() as ctx:
        const = ctx.enter_context(tc.tile_pool(name="const", bufs=1))

        s_t = const.tile([128, 1], f32)
        nc.gpsimd.dma_start(out=s_t[:], in_=rw[:])
        nc.vector.tensor_scalar_mul(s_t[:], s_t[:], 0.1)   # s = 0.1 * reward

        va_t = const.tile([R, D], f32)
        nc.gpsimd.dma_start(out=va_t[:], in_=va[:])
        nc.scalar.mul(va_t[:], va_t[:], 0.1)               # 0.1 * V folded in
        v1_t = const.tile([R, D], f32)
        nc.gpsimd.dma_start(out=v1_t[:], in_=v1[:])
        nc.scalar.mul(v1_t[:], v1_t[:], 0.1)
        v2_t = const.tile([R, F], f32)
        nc.gpsimd.dma_start(out=v2_t[:], in_=v2[:])
        nc.scalar.mul(v2_t[:], v2_t[:], 0.1)

        uta_t = const.tile([R, RA], f32)
        nc.gpsimd.dma_start(out=uta_t[:], in_=uta[:])
        ut1_t = const.tile([R, R1], f32)
        nc.gpsimd.dma_start(out=ut1_t[:], in_=ut1[:])
        ut2_t = const.tile([R, R2], f32)
        nc.gpsimd.dma_start(out=ut2_t[:], in_=ut2[:])

        gp = ctx.enter_context(tc.tile_pool(name="gp", bufs=4))
        tp = ctx.enter_context(tc.tile_pool(name="tp", bufs=4))
        mp = ctx.enter_context(tc.tile_pool(name="mp", bufs=4))
        dp = ctx.enter_context(tc.tile_pool(name="dp", bufs=4))
        pp = ctx.enter_context(tc.tile_pool(name="pp", bufs=2, space="PSUM"))

        def position(gd, td, md, r0, c0, ut_t, v_t, od_t, od_m, od_d):
            rs, cs = slice(r0, r0 + 128), slice(c0, c0 + TILE_N)
            gt = gp.tile([128, TILE_N], f32)
            nc.sync.dma_start(out=gt[:], in_=gd[rs, cs])
            tt = tp.tile([128, TILE_N], f32)
            nc.sync.dma_start(out=tt[:], in_=td[rs, cs])
            mt = mp.tile([128, TILE_N], f32)
            nc.sync.dma_start(out=mt[:], in_=md[rs, cs])

            nc.scalar.mul(gt[:], gt[:], 0.1)                       # 0.1*g
            nc.vector.scalar_tensor_tensor(                        # t' = 0.9t + 0.1g
                tt[:], tt[:], 0.9, gt[:], op0=MULT, op1=ADD)
            nc.scalar.dma_start(out=od_t[rs, cs], in_=tt[:])
            nc.scalar.mul(gt[:], tt[:], 0.05)                      # 0.05*t'
            nc.vector.scalar_tensor_tensor(                        # m' = 0.95m + 0.05t'
                mt[:], mt[:], 0.95, gt[:], op0=MULT, op1=ADD)
            nc.scalar.dma_start(out=od_m[rs, cs], in_=mt[:])

            ps = pp.tile([128, TILE_N], f32)
            for j in range(TILE_N // MM_N):
                nc.tensor.matmul(                                  # 0.1*(U@V) chunk
                    ps[:, j * MM_N:(j + 1) * MM_N],
                    lhsT=ut_t[:, r0:r0 + 128],
                    rhs=v_t[:, c0 + j * MM_N:c0 + (j + 1) * MM_N],
                    start=True, stop=True)
            dt = dp.tile([128, TILE_N], f32)
            nc.vector.scalar_tensor_tensor(                        # mod = s*m' + UV
                dt[:], mt[:], s_t[:, 0:1], ps[:], op0=MULT, op1=ADD)
            nc.scalar.dma_start(out=od_d[rs, cs], in_=dt[:])

        for r0 in range(0, RA, 128):
            position(ga, ta, ma, r0, 0, uta_t, va_t, ota_t, ota_m, ota_d)
        for r0 in range(0, R1, 128):
            position(g1, t1, m1, r0, 0, ut1_t, v1_t, o1_t, o1_m, o1_d)
        for r0 in range(0, R2, 128):
            for c0 in range(0, F, TILE_N):
                position(g2, t2, m2, r0, c0, ut2_t, v2_t, o2_t, o2_m, o2_d)

        # bias shard [8, 128] -- no low-rank term
        gt = gp.tile([8, 128], f32, tag="gt")
        nc.sync.dma_start(out=gt[:], in_=gb[:])
        tt = tp.tile([8, 128], f32, tag="tt")
        nc.sync.dma_start(out=tt[:], in_=tb[:])
        mt = mp.tile([8, 128], f32, tag="mt")
        nc.sync.dma_start(out=mt[:], in_=mb[:])
        nc.scalar.mul(gt[:], gt[:], 0.1)
        nc.vector.scalar_tensor_tensor(tt[:], tt[:], 0.9, gt[:], op0=MULT, op1=ADD)
        nc.scalar.dma_start(out=ob_t[:], in_=tt[:])
        nc.scalar.mul(gt[:], tt[:], 0.05)
        nc.vector.scalar_tensor_tensor(mt[:], mt[:], 0.95, gt[:], op0=MULT, op1=ADD)
        nc.scalar.dma_start(out=ob_m[:], in_=mt[:])
        dt = dp.tile([8, 128], f32, tag="dt")
        nc.vector.tensor_scalar_mul(dt[:], mt[:], s_t[0:8, 0:1])
        nc.scalar.dma_start(out=ob_d[:], in_=dt[:])

    _split_multi_waits(nc)
    _BUILT["nc"] = nc
    return nc


def _shard_inputs(inputs):
    """Full inputs -> one in_map per core (row-sharded, views where possible)."""
    c32 = lambda a: np.ascontiguousarray(a, dtype=np.float32)
    rw = c32(np.broadcast_to(inputs["reward_signal"].reshape(1, 1), (128, 1)))
    in_maps = []
    for c in range(NCORES):
        sa = slice(c * RA, (c + 1) * RA)
        s1 = slice(c * R1, (c + 1) * R1)
        s2 = slice(c * R2, (c + 1) * R2)
        sb = slice(c * RB, (c + 1) * RB)
        in_maps.append({
            "ga": inputs["grad_attn"][sa], "ta": inputs["trace_attn"][sa],
            "ma": inputs["mom_attn"][sa],
            "uta": c32(inputs["U_attn"][sa].T), "va": inputs["V_attn"],
            "g1": inputs["grad_ff1"][s1], "t1": inputs["trace_ff1"][s1],
            "m1": inputs["mom_ff1"][s1],
            "ut1": c32(inputs["U_ff1"][s1].T), "v1": inputs["V_ff1"],
            "g2": inputs["grad_ff2"][s2], "t2": inputs["trace_ff2"][s2],
            "m2": inputs["mom_ff2"][s2],
            "ut2": c32(inputs["U_ff2"][s2].T), "v2": inputs["V_ff2"],
            "gb": inputs["grad_b"][sb].reshape(8, 128),
            "tb": inputs["trace_b"][sb].reshape(8, 128),
            "mb": inputs["mom_b"][sb].reshape(8, 128),
            "rw": rw,
        })
    return in_maps


def _gather(results):
    cat = lambda key: np.concatenate([results[c][key] for c in range(NCORES)], axis=0)
    catb = lambda key: np.concatenate(
        [results[c][key].reshape(RB) for c in range(NCORES)], axis=0)
    return (
        cat("ota_t"), cat("ota_m"), cat("ota_d"),
        cat("o1_t"), cat("o1_m"), cat("o1_d"),
        cat("o2_t"), cat("o2_m"), cat("o2_d"),
        catb("ob_t"), catb("ob_m"), catb("ob_d"),
    )


def run_sharded(inputs, trace=False, tmpdir=None):
    """Build + run; returns (outputs_tuple, BassKernelResults)."""
    nc = _build_program()
    import concourse.bass_utils as bass_utils
    in_maps = _shard_inputs(inputs)
    res = bass_utils.run_bass_kernel_spmd(
        nc, in_maps, list(range(NCORES)), trace=trace, tmpdir=tmpdir)
    return _gather(res.results), res


def kernel(**inputs):
    outputs, _ = run_sharded(inputs, trace=False)
    return outputs


# revision 8
# speedup vs baseline: 1.4222x; 1.0008x over previous
"""Trainium2 Bass kernel for the NeuroPlasticityMechanism problem.

Row-sharded tensor-parallel across 8 NeuronCores (no communication):
each core gets a contiguous row-block of every large tensor plus the full
(small) V factors.  Per element the kernel computes

    t' = 0.9*t + 0.1*g
    m' = 0.95*m + 0.05*t'
    mod = (0.1*reward)*m' + 0.1*(U@V)        (matrix params)
    mod = (0.1*reward)*m'                    (bias)

streaming [128 x 2048] fp32 tiles: 2 ScalarE scale ops + 3 VectorE fused
scalar_tensor_tensor ops per tile, U@V on the PE from host-pretransposed
U^T slices, everything overlapped with HWDGE DMA.  The kernel is
HBM-bandwidth-bound (~109 MB of DMA traffic per core).
"""
import sys
import types
from contextlib import ExitStack

import numpy as np

D, F, R = 2048, 8192, 64
NCORES = 8
RA = D // NCORES      # 256 attn rows per core
R1 = F // NCORES      # 1024 ff1 rows per core
R2 = D // NCORES      # 256 ff2 rows per core
RB = F // NCORES      # 1024 bias elems per core
TILE_N = 2048         # free-dim stream tile
MM_N = 512            # fp32 matmul moving-operand max / one PSUM bank


def _install_tile_drain_patch():
    """This walrus build caps sync-wait commands on TPB_CTRL (Drain)
    instructions below what Tile's tail drain emits.  Re-emit the waits as
    individual wait_ge instructions on the sync engine, then a bare drain."""
    from concourse import tile
    from concourse.vector_clock import ScopedClock

    if getattr(tile.TileContext, "_drain_patch_installed", False):
        return

    def _drain_and_barrier(self, tick_clock, wait_clock):
        nc = self.nc
        probe = nc.sync.nop(nofuse=True)
        wait_clock.add_sem_waits(
            probe.ins, ScopedClock({None: tick_clock.global_clock})
        )
        si = probe.ins.sync_info
        waits = list(si.on_wait or []) if si else []
        if si:
            si.on_wait = []
        handles = {h.name: h for h in self.sems.allocated().values()}
        # Spread the waits across compute engines so they retire in
        # parallel (~85ns each serially); the barrier below joins them.
        wait_engines = [nc.sync, nc.vector, nc.scalar, nc.tensor]
        for i, w in enumerate(waits):
            assert w.wait_mode == "sem-ge-imm", w
            wait_engines[i % len(wait_engines)].wait_ge(
                handles[w.ant_name], w.wait_value)
        nc.sync.drain()
        nc.all_engine_barrier()
        popped = nc._tile_sem_poison_stack.pop()
        assert popped is self._sem_poison
        nc.clear_and_free_semaphores(list(self.sems.allocated().values()))
        nc.all_engine_barrier()

    tile.TileContext._drain_and_barrier = _drain_and_barrier
    tile.TileContext._drain_patch_installed = True


def _split_multi_waits(nc):
    """This stack's walrus accepts only one sync-wait command per
    instruction, but Tile's scheduler attaches several.  Hoist all but the
    last wait of each instruction onto dedicated EventSemaphore wait
    instructions inserted just before it on the same engine queue."""
    from concourse import mybir

    n = 0
    for f in nc.m.functions:
        for bb in f.blocks:
            out = []
            changed = False
            for inst in bb.instructions:
                si = inst.sync_info
                if si is not None and si.on_wait and len(si.on_wait) > 1:
                    waits = list(si.on_wait)
                    for w in waits[:-1]:
                        n += 1
                        out.append(mybir.InstEventSemaphore(
                            name=f"I-waitsplit-{n}", ins=[], outs=[],
                            engine=inst.engine,
                            sync_info=mybir.SyncInfo(on_wait=[w], on_update=[]),
                        ))
                    si.on_wait = [waits[-1]]
                    changed = True
                out.append(inst)
            if changed:
                bb.instructions = out


_BUILT = {}


def _build_program():
    if "nc" in _BUILT:
        return _BUILT["nc"]

    _install_tile_drain_patch()
    import concourse.bass as bass
    import concourse.tile as tile
    from concourse import mybir

    f32 = mybir.dt.float32
    MULT = mybir.AluOpType.mult
    ADD = mybir.AluOpType.add

    nc = bass.Bass()

    def inp(name, shape):
        return nc.declare_dram_parameter(name, list(shape), f32, isOutput=False)

    def outp(name, shape):
        return nc.declare_dram_parameter(name, list(shape), f32, isOutput=True)

    ga, ta, ma = (inp(n, (RA, D)) for n in ("ga", "ta", "ma"))
    uta = inp("uta", (R, RA))          # U_attn shard, pre-transposed
    va = inp("va", (R, D))
    g1, t1, m1 = (inp(n, (R1, D)) for n in ("g1", "t1", "m1"))
    ut1 = inp("ut1", (R, R1))
    v1 = inp("v1", (R, D))
    g2, t2, m2 = (inp(n, (R2, F)) for n in ("g2", "t2", "m2"))
    ut2 = inp("ut2", (R, R2))
    v2 = inp("v2", (R, F))
    gb, tb, mb = (inp(n, (8, 128)) for n in ("gb", "tb", "mb"))
    rw = inp("rw", (128, 1))

    ota_t, ota_m, ota_d = (outp(n, (RA, D)) for n in ("ota_t", "ota_m", "ota_d"))
    o1_t, o1_m, o1_d = (outp(n, (R1, D)) for n in ("o1_t", "o1_m", "o1_d"))
    o2_t, o2_m, o2_d = (outp(n, (R2, F)) for n in ("o2_t", "o2_m", "o2_d"))
    ob_t, ob_m, ob_d = (outp(n, (8, 128)) for n in ("ob_t", "ob_m", "ob_d"))

    with tile.TileContext(nc) as tc, ExitStack() as ctx:
        const = ctx.enter_context(tc.tile_pool(name="const", bufs=1))

        s_t = const.tile([128, 1], f32)
        nc.gpsimd.dma_start(out=s_t[:], in_=rw[:])
        nc.vector.tensor_scalar_mul(s_t[:], s_t[:], 0.1)   # s = 0.1 * reward

        # 0.1*(U@V) is computed as (0.1*U^T)^T @ V -- prescaling the small
        # U^T factors (0.4 MB) instead of V (3 MB) keeps the ScalarE queue
        # free so position-0 stores can issue early.
        uta_t = const.tile([R, RA], f32)
        nc.gpsimd.dma_start(out=uta_t[:], in_=uta[:])
        nc.scalar.mul(uta_t[:], uta_t[:], 0.1)
        ut1_t = const.tile([R, R1], f32)
        nc.gpsimd.dma_start(out=ut1_t[:], in_=ut1[:])
        nc.scalar.mul(ut1_t[:], ut1_t[:], 0.1)
        ut2_t = const.tile([R, R2], f32)
        nc.gpsimd.dma_start(out=ut2_t[:], in_=ut2[:])
        nc.scalar.mul(ut2_t[:], ut2_t[:], 0.1)

        va_t = const.tile([R, D], f32)
        nc.gpsimd.dma_start(out=va_t[:], in_=va[:])
        v1_t = const.tile([R, D], f32)
        nc.gpsimd.dma_start(out=v1_t[:], in_=v1[:])
        v2_t = const.tile([R, F], f32)
        nc.gpsimd.dma_start(out=v2_t[:], in_=v2[:])

        gp = ctx.enter_context(tc.tile_pool(name="gp", bufs=4))
        tp = ctx.enter_context(tc.tile_pool(name="tp", bufs=4))
        mp = ctx.enter_context(tc.tile_pool(name="mp", bufs=4))
        dp = ctx.enter_context(tc.tile_pool(name="dp", bufs=4))
        pp = ctx.enter_context(tc.tile_pool(name="pp", bufs=2, space="PSUM"))

        def position(gd, td, md, r0, c0, ut_t, v_t, od_t, od_m, od_d):
            rs, cs = slice(r0, r0 + 128), slice(c0, c0 + TILE_N)
            gt = gp.tile([128, TILE_N], f32)
            nc.sync.dma_start(out=gt[:], in_=gd[rs, cs])
            tt = tp.tile([128, TILE_N], f32)
            nc.sync.dma_start(out=tt[:], in_=td[rs, cs])
            mt = mp.tile([128, TILE_N], f32)
            nc.sync.dma_start(out=mt[:], in_=md[rs, cs])

            nc.scalar.mul(gt[:], gt[:], 0.1)                       # 0.1*g
            nc.vector.scalar_tensor_tensor(                        # t' = 0.9t + 0.1g
                tt[:], tt[:], 0.9, gt[:], op0=MULT, op1=ADD)
            nc.scalar.dma_start(out=od_t[rs, cs], in_=tt[:])
            nc.scalar.mul(gt[:], tt[:], 0.05)                      # 0.05*t'
            nc.vector.scalar_tensor_tensor(                        # m' = 0.95m + 0.05t'
                mt[:], mt[:], 0.95, gt[:], op0=MULT, op1=ADD)
            nc.scalar.dma_start(out=od_m[rs, cs], in_=mt[:])

            ps = pp.tile([128, TILE_N], f32)
            for j in range(TILE_N // MM_N):
                nc.tensor.matmul(                                  # 0.1*(U@V) chunk
                    ps[:, j * MM_N:(j + 1) * MM_N],
                    lhsT=ut_t[:, r0:r0 + 128],
                    rhs=v_t[:, c0 + j * MM_N:c0 + (j + 1) * MM_N],
                    start=True, stop=True)
            dt = dp.tile([128, TILE_N], f32)
            nc.vector.scalar_tensor_tensor(                        # mod = s*m' + UV
                dt[:], mt[:], s_t[:, 0:1], ps[:], op0=MULT, op1=ADD)
            nc.scalar.dma_start(out=od_d[rs, cs], in_=dt[:])

        for r0 in range(0, RA, 128):
            position(ga, ta, ma, r0, 0, uta_t, va_t, ota_t, ota_m, ota_d)
        for r0 in range(0, R1, 128):
            position(g1, t1, m1, r0, 0, ut1_t, v1_t, o1_t, o1_m, o1_d)
        for r0 in range(0, R2, 128):
            for c0 in range(0, F, TILE_N):
                position(g2, t2, m2, r0, c0, ut2_t, v2_t, o2_t, o2_m, o2_d)

        # bias shard [8, 128] -- no low-rank term
        gt = gp.tile([8, 128], f32, tag="gt")
        nc.sync.dma_start(out=gt[:], in_=gb[:])
        tt = tp.tile([8, 128], f32, tag="tt")
        nc.sync.dma_start(out=tt[:], in_=tb[:])
        mt = mp.tile([8, 128], f32, tag="mt")
        nc.sync.dma_start(out=mt[:], in_=mb[:])
        nc.scalar.mul(gt[:], gt[:], 0.1)
        nc.vector.scalar_tensor_tensor(tt[:], tt[:], 0.9, gt[:], op0=MULT, op1=ADD)
        nc.scalar.dma_start(out=ob_t[:], in_=tt[:])
        nc.scalar.mul(gt[:], tt[:], 0.05)
        nc.vector.scalar_tensor_tensor(mt[:], mt[:], 0.95, gt[:], op0=MULT, op1=ADD)
        nc.scalar.dma_start(out=ob_m[:], in_=mt[:])
        dt = dp.tile([8, 128], f32, tag="dt")
        nc.vector.tensor_scalar_mul(dt[:], mt[:], s_t[0:8, 0:1])
        nc.scalar.dma_start(out=ob_d[:], in_=dt[:])

    _split_multi_waits(nc)
    _BUILT["nc"] = nc
    return nc


def _shard_inputs(inputs):
    """Full inputs -> one in_map per core (row-sharded, views where possible)."""
    c32 = lambda a: np.ascontiguousarray(a, dtype=np.float32)
    rw = c32(np.broadcast_to(inputs["reward_signal"].reshape(1, 1), (128, 1)))
    in_maps = []
    for c in range(NCORES):
        sa = slice(c * RA, (c + 1) * RA)
        s1 = slice(c * R1, (c + 1) * R1)
        s2 = slice(c * R2, (c + 1) * R2)
        sb = slice(c * RB, (c + 1) * RB)
        in_maps.append({
            "ga": inputs["grad_attn"][sa], "ta": inputs["trace_attn"][sa],
            "ma": inputs["mom_attn"][sa],
            "uta": c32(inputs["U_attn"][sa].T), "va": inputs["V_attn"],
            "g1": inputs["grad_ff1"][s1], "t1": inputs["trace_ff1"][s1],
            "m1": inputs["mom_ff1"][s1],
            "ut1": c32(inputs["U_ff1"][s1].T), "v1": inputs["V_ff1"],
            "g2": inputs["grad_ff2"][s2], "t2": inputs["trace_ff2"][s2],
            "m2": inputs["mom_ff2"][s2],
            "ut2": c32(inputs["U_ff2"][s2].T), "v2": inputs["V_ff2"],
            "gb": inputs["grad_b"][sb].reshape(8, 128),
            "tb": inputs["trace_b"][sb].reshape(8, 128),
            "mb": inputs["mom_b"][sb].reshape(8, 128),
            "rw": rw,
        })
    return in_maps


def _gather(results):
    cat = lambda key: np.concatenate([results[c][key] for c in range(NCORES)], axis=0)
    catb = lambda key: np.concatenate(
        [results[c][key].reshape(RB) for c in range(NCORES)], axis=0)
    return (
        cat("ota_t"), cat("ota_m"), cat("ota_d"),
        cat("o1_t"), cat("o1_m"), cat("o1_d"),
        cat("o2_t"), cat("o2_m"), cat("o2_d"),
        catb("ob_t"), catb("ob_m"), catb("ob_d"),
    )


def run_sharded(inputs, trace=False, tmpdir=None):
    """Build + run; returns (outputs_tuple, BassKernelResults)."""
    nc = _build_program()
    import concourse.bass_utils as bass_utils
    in_maps = _shard_inputs(inputs)
    res = bass_utils.run_bass_kernel_spmd(
        nc, in_maps, list(range(NCORES)), trace=trace, tmpdir=tmpdir)
    return _gather(res.results), res


def kernel(**inputs):
    outputs, _ = run_sharded(inputs, trace=False)
    return outputs


# revision 9
# speedup vs baseline: 1.4515x; 1.0206x over previous
"""Trainium2 Bass kernel for the NeuroPlasticityMechanism problem.

Row-sharded tensor-parallel across 8 NeuronCores (no communication):
each core gets a contiguous row-block of every large tensor plus the full
(small) V factors.  Per element the kernel computes

    t' = 0.9*t + 0.1*g
    m' = 0.95*m + 0.05*t'
    mod = (0.1*reward)*m' + 0.1*(U@V)        (matrix params)
    mod = (0.1*reward)*m'                    (bias)

streaming [128 x 2048] fp32 tiles: 2 ScalarE scale ops + 3 VectorE fused
scalar_tensor_tensor ops per tile, U@V on the PE from host-pretransposed
U^T slices, everything overlapped with HWDGE DMA.  The kernel is
HBM-bandwidth-bound (~109 MB of DMA traffic per core).
"""
import sys
import types
from contextlib import ExitStack

import numpy as np

D, F, R = 2048, 8192, 64
NCORES = 8
RA = D // NCORES      # 256 attn rows per core
R1 = F // NCORES      # 1024 ff1 rows per core
R2 = D // NCORES      # 256 ff2 rows per core
RB = F // NCORES      # 1024 bias elems per core
TILE_N = 2048         # free-dim stream tile
MM_N = 512            # fp32 matmul moving-operand max / one PSUM bank


def _install_tile_drain_patch():
    """This walrus build caps sync-wait commands on TPB_CTRL (Drain)
    instructions below what Tile's tail drain emits.  Re-emit the waits as
    individual wait_ge instructions on the sync engine, then a bare drain."""
    from concourse import tile
    from concourse.vector_clock import ScopedClock

    if getattr(tile.TileContext, "_drain_patch_installed", False):
        return

    def _drain_and_barrier(self, tick_clock, wait_clock):
        nc = self.nc
        probe = nc.sync.nop(nofuse=True)
        wait_clock.add_sem_waits(
            probe.ins, ScopedClock({None: tick_clock.global_clock})
        )
        si = probe.ins.sync_info
        waits = list(si.on_wait or []) if si else []
        if si:
            si.on_wait = []
        handles = {h.name: h for h in self.sems.allocated().values()}
        # Spread the waits across compute engines so they retire in
        # parallel (~85ns each serially); the barrier below joins them.
        wait_engines = [nc.sync, nc.vector, nc.scalar, nc.tensor]
        for i, w in enumerate(waits):
            assert w.wait_mode == "sem-ge-imm", w
            wait_engines[i % len(wait_engines)].wait_ge(
                handles[w.ant_name], w.wait_value)
        nc.sync.drain()
        nc.all_engine_barrier()
        popped = nc._tile_sem_poison_stack.pop()
        assert popped is self._sem_poison
        nc.clear_and_free_semaphores(list(self.sems.allocated().values()))
        nc.all_engine_barrier()

    tile.TileContext._drain_and_barrier = _drain_and_barrier
    tile.TileContext._drain_patch_installed = True


def _split_multi_waits(nc):
    """This stack's walrus accepts only one sync-wait command per
    instruction, but Tile's scheduler attaches several.  Hoist all but the
    last wait of each instruction onto dedicated EventSemaphore wait
    instructions inserted just before it on the same engine queue."""
    from concourse import mybir

    n = 0
    for f in nc.m.functions:
        for bb in f.blocks:
            out = []
            changed = False
            for inst in bb.instructions:
                si = inst.sync_info
                if si is not None and si.on_wait and len(si.on_wait) > 1:
                    waits = list(si.on_wait)
                    for w in waits[:-1]:
                        n += 1
                        out.append(mybir.InstEventSemaphore(
                            name=f"I-waitsplit-{n}", ins=[], outs=[],
                            engine=inst.engine,
                            sync_info=mybir.SyncInfo(on_wait=[w], on_update=[]),
                        ))
                    si.on_wait = [waits[-1]]
                    changed = True
                out.append(inst)
            if changed:
                bb.instructions = out


_BUILT = {}


def _build_program():
    if "nc" in _BUILT:
        return _BUILT["nc"]

    _install_tile_drain_patch()
    import concourse.bass as bass
    import concourse.tile as tile
    from concourse import mybir

    f32 = mybir.dt.float32
    MULT = mybir.AluOpType.mult
    ADD = mybir.AluOpType.add

    nc = bass.Bass()

    def inp(name, shape):
        return nc.declare_dram_parameter(name, list(shape), f32, isOutput=False)

    def outp(name, shape):
        return nc.declare_dram_parameter(name, list(shape), f32, isOutput=True)

    ga, ta, ma = (inp(n, (RA, D)) for n in ("ga", "ta", "ma"))
    uta = inp("uta", (R, RA))          # U_attn shard, pre-transposed
    va = inp("va", (R, D))
    g1, t1, m1 = (inp(n, (R1, D)) for n in ("g1", "t1", "m1"))
    ut1 = inp("ut1", (R, R1))
    v1 = inp("v1", (R, D))
    g2, t2, m2 = (inp(n, (R2, F)) for n in ("g2", "t2", "m2"))
    ut2 = inp("ut2", (R, R2))
    v2 = inp("v2", (R, F))
    gb, tb, mb = (inp(n, (8, 128)) for n in ("gb", "tb", "mb"))
    rw = inp("rw", (128, 1))

    ota_t, ota_m, ota_d = (outp(n, (RA, D)) for n in ("ota_t", "ota_m", "ota_d"))
    o1_t, o1_m, o1_d = (outp(n, (R1, D)) for n in ("o1_t", "o1_m", "o1_d"))
    o2_t, o2_m, o2_d = (outp(n, (R2, F)) for n in ("o2_t", "o2_m", "o2_d"))
    ob_t, ob_m, ob_d = (outp(n, (8, 128)) for n in ("ob_t", "ob_m", "ob_d"))

    with tile.TileContext(nc) as tc, ExitStack() as ctx:
        const = ctx.enter_context(tc.tile_pool(name="const", bufs=1))

        # Small consts (reward + pre-transposed U^T, 0.4 MB) go first on the
        # fast sync HWDGE ring so the ScalarE prescales unblock early; the
        # big raw V factors (3 MB, not needed until the matmuls) ride the
        # otherwise-idle GpSimd SWDGE ring in parallel with the stream.
        s_t = const.tile([128, 1], f32)
        nc.sync.dma_start(out=s_t[:], in_=rw[:])
        nc.vector.tensor_scalar_mul(s_t[:], s_t[:], 0.1)   # s = 0.1 * reward

        # 0.1*(U@V) is computed as (0.1*U^T)^T @ V -- prescaling the small
        # U^T factors instead of V keeps the ScalarE queue free so
        # position-0 stores can issue early.
        uta_t = const.tile([R, RA], f32)
        nc.sync.dma_start(out=uta_t[:], in_=uta[:])
        nc.scalar.mul(uta_t[:], uta_t[:], 0.1)
        ut1_t = const.tile([R, R1], f32)
        nc.sync.dma_start(out=ut1_t[:], in_=ut1[:])
        nc.scalar.mul(ut1_t[:], ut1_t[:], 0.1)
        ut2_t = const.tile([R, R2], f32)
        nc.sync.dma_start(out=ut2_t[:], in_=ut2[:])
        nc.scalar.mul(ut2_t[:], ut2_t[:], 0.1)

        va_t = const.tile([R, D], f32)
        nc.gpsimd.dma_start(out=va_t[:], in_=va[:])
        v1_t = const.tile([R, D], f32)
        nc.gpsimd.dma_start(out=v1_t[:], in_=v1[:])
        v2_t = const.tile([R, F], f32)
        nc.gpsimd.dma_start(out=v2_t[:], in_=v2[:])

        gp = ctx.enter_context(tc.tile_pool(name="gp", bufs=4))
        tp = ctx.enter_context(tc.tile_pool(name="tp", bufs=4))
        mp = ctx.enter_context(tc.tile_pool(name="mp", bufs=4))
        dp = ctx.enter_context(tc.tile_pool(name="dp", bufs=4))
        pp = ctx.enter_context(tc.tile_pool(name="pp", bufs=2, space="PSUM"))

        def position(gd, td, md, r0, c0, ut_t, v_t, od_t, od_m, od_d):
            rs, cs = slice(r0, r0 + 128), slice(c0, c0 + TILE_N)
            gt = gp.tile([128, TILE_N], f32)
            nc.sync.dma_start(out=gt[:], in_=gd[rs, cs])
            tt = tp.tile([128, TILE_N], f32)
            nc.sync.dma_start(out=tt[:], in_=td[rs, cs])
            mt = mp.tile([128, TILE_N], f32)
            nc.sync.dma_start(out=mt[:], in_=md[rs, cs])

            nc.scalar.mul(gt[:], gt[:], 0.1)                       # 0.1*g
            nc.vector.scalar_tensor_tensor(                        # t' = 0.9t + 0.1g
                tt[:], tt[:], 0.9, gt[:], op0=MULT, op1=ADD)
            nc.scalar.dma_start(out=od_t[rs, cs], in_=tt[:])
            nc.scalar.mul(gt[:], tt[:], 0.05)                      # 0.05*t'
            nc.vector.scalar_tensor_tensor(                        # m' = 0.95m + 0.05t'
                mt[:], mt[:], 0.95, gt[:], op0=MULT, op1=ADD)
            nc.scalar.dma_start(out=od_m[rs, cs], in_=mt[:])

            ps = pp.tile([128, TILE_N], f32)
            for j in range(TILE_N // MM_N):
                nc.tensor.matmul(                                  # 0.1*(U@V) chunk
                    ps[:, j * MM_N:(j + 1) * MM_N],
                    lhsT=ut_t[:, r0:r0 + 128],
                    rhs=v_t[:, c0 + j * MM_N:c0 + (j + 1) * MM_N],
                    start=True, stop=True)
            dt = dp.tile([128, TILE_N], f32)
            nc.vector.scalar_tensor_tensor(                        # mod = s*m' + UV
                dt[:], mt[:], s_t[:, 0:1], ps[:], op0=MULT, op1=ADD)
            nc.scalar.dma_start(out=od_d[rs, cs], in_=dt[:])

        for r0 in range(0, RA, 128):
            position(ga, ta, ma, r0, 0, uta_t, va_t, ota_t, ota_m, ota_d)
        for r0 in range(0, R1, 128):
            position(g1, t1, m1, r0, 0, ut1_t, v1_t, o1_t, o1_m, o1_d)
        for r0 in range(0, R2, 128):
            for c0 in range(0, F, TILE_N):
                position(g2, t2, m2, r0, c0, ut2_t, v2_t, o2_t, o2_m, o2_d)

        # bias shard [8, 128] -- no low-rank term
        gt = gp.tile([8, 128], f32, tag="gt")
        nc.sync.dma_start(out=gt[:], in_=gb[:])
        tt = tp.tile([8, 128], f32, tag="tt")
        nc.sync.dma_start(out=tt[:], in_=tb[:])
        mt = mp.tile([8, 128], f32, tag="mt")
        nc.sync.dma_start(out=mt[:], in_=mb[:])
        nc.scalar.mul(gt[:], gt[:], 0.1)
        nc.vector.scalar_tensor_tensor(tt[:], tt[:], 0.9, gt[:], op0=MULT, op1=ADD)
        nc.scalar.dma_start(out=ob_t[:], in_=tt[:])
        nc.scalar.mul(gt[:], tt[:], 0.05)
        nc.vector.scalar_tensor_tensor(mt[:], mt[:], 0.95, gt[:], op0=MULT, op1=ADD)
        nc.scalar.dma_start(out=ob_m[:], in_=mt[:])
        dt = dp.tile([8, 128], f32, tag="dt")
        nc.vector.tensor_scalar_mul(dt[:], mt[:], s_t[0:8, 0:1])
        nc.scalar.dma_start(out=ob_d[:], in_=dt[:])

    _split_multi_waits(nc)
    _BUILT["nc"] = nc
    return nc


def _shard_inputs(inputs):
    """Full inputs -> one in_map per core (row-sharded, views where possible)."""
    c32 = lambda a: np.ascontiguousarray(a, dtype=np.float32)
    rw = c32(np.broadcast_to(inputs["reward_signal"].reshape(1, 1), (128, 1)))
    in_maps = []
    for c in range(NCORES):
        sa = slice(c * RA, (c + 1) * RA)
        s1 = slice(c * R1, (c + 1) * R1)
        s2 = slice(c * R2, (c + 1) * R2)
        sb = slice(c * RB, (c + 1) * RB)
        in_maps.append({
            "ga": inputs["grad_attn"][sa], "ta": inputs["trace_attn"][sa],
            "ma": inputs["mom_attn"][sa],
            "uta": c32(inputs["U_attn"][sa].T), "va": inputs["V_attn"],
            "g1": inputs["grad_ff1"][s1], "t1": inputs["trace_ff1"][s1],
            "m1": inputs["mom_ff1"][s1],
            "ut1": c32(inputs["U_ff1"][s1].T), "v1": inputs["V_ff1"],
            "g2": inputs["grad_ff2"][s2], "t2": inputs["trace_ff2"][s2],
            "m2": inputs["mom_ff2"][s2],
            "ut2": c32(inputs["U_ff2"][s2].T), "v2": inputs["V_ff2"],
            "gb": inputs["grad_b"][sb].reshape(8, 128),
            "tb": inputs["trace_b"][sb].reshape(8, 128),
            "mb": inputs["mom_b"][sb].reshape(8, 128),
            "rw": rw,
        })
    return in_maps


def _gather(results):
    cat = lambda key: np.concatenate([results[c][key] for c in range(NCORES)], axis=0)
    catb = lambda key: np.concatenate(
        [results[c][key].reshape(RB) for c in range(NCORES)], axis=0)
    return (
        cat("ota_t"), cat("ota_m"), cat("ota_d"),
        cat("o1_t"), cat("o1_m"), cat("o1_d"),
        cat("o2_t"), cat("o2_m"), cat("o2_d"),
        catb("ob_t"), catb("ob_m"), catb("ob_d"),
    )


def run_sharded(inputs, trace=False, tmpdir=None):
    """Build + run; returns (outputs_tuple, BassKernelResults)."""
    nc = _build_program()
    import concourse.bass_utils as bass_utils
    in_maps = _shard_inputs(inputs)
    res = bass_utils.run_bass_kernel_spmd(
        nc, in_maps, list(range(NCORES)), trace=trace, tmpdir=tmpdir)
    return _gather(res.results), res


def kernel(**inputs):
    outputs, _ = run_sharded(inputs, trace=False)
    return outputs
